# revision 1
# baseline (speedup 1.0000x reference)
"""Trainium2 Bass kernel for 16-head MHA (B=2, T=2048, C=1024).

Sharding: 8 cores = 2 batches x 4 head-groups (4 heads each).
Each core computes, for its batch b and head group g:
  partialT[c, t] = sum_{h in g} wo[:, h].T @ (softmax(qk^T) @ v_h)^T
in fully transposed space (no on-device transposes needed):
  - host passes xT = x[b].T and pre-transposed weight slices
  - qT/kT computed as [d, t]; v as [t, d] (+ ones column per head for the
    softmax denominator); scores computed directly as [tk, tq]
  - exp applied with per-partition (key) mask bias; denominator emerges as
    row 64 of the attn@v_ext matmul output; normalization folded in as a
    K=1 "replicate" matmul + elementwise multiply
  - final projection consumes the [d, t] head outputs as stationary weights
Host adds the 4 partial sums per batch, the wo bias, and the wv_b @ wo.T
constant row (v-bias contribution commutes through softmax normalization).
"""

import sys

sys.path.insert(0, "/opt/trn_rl_repo")

import numpy as np

# ---- problem constants (hardcoded per harness contract) ----
B = 2
T = 2048
C = 1024
NUM_HEADS = 16
G = 4                 # head groups (tensor-parallel dimension)
HPG = NUM_HEADS // G  # 4 heads per core
DH = C // NUM_HEADS   # 64
DC = HPG * DH         # 256 dims per core
VE = HPG * (DH + 1)   # 260: per head 64 v-dims + 1 ones column
N_CORES = B * G       # 8
PAD_ID = 0.0

CH = 512              # tq chunk (one PSUM bank of fp32)
NCH = T // CH         # 4
NT = T // 128         # 16 token tiles
KT = C // 128         # 8 contraction tiles for projections
DM = DC // 128        # 2 m-tiles for q/k


def build_nc(debug=False):
    import concourse.tile as tile
    from concourse import bacc, mybir

    f32 = mybir.dt.float32
    f32r = mybir.dt.float32r
    f16 = mybir.dt.float16
    Exp = mybir.ActivationFunctionType.Exp
    is_equal = mybir.AluOpType.is_equal
    mult = mybir.AluOpType.mult

    nc = bacc.Bacc(
        "TRN2", target_bir_lowering=False, debug=debug, num_devices=N_CORES
    )

    xT_d = nc.dram_tensor("xT", [C, T], f16, kind="ExternalInput")
    wqT_d = nc.dram_tensor("wqT", [C, DC], f16, kind="ExternalInput")
    wkT_d = nc.dram_tensor("wkT", [C, DC], f16, kind="ExternalInput")
    wvT_d = nc.dram_tensor("wvT", [C, VE], f16, kind="ExternalInput")
    woT_d = nc.dram_tensor("woT", [DC, C], f16, kind="ExternalInput")
    bq_d = nc.dram_tensor("bq", [DC], f32, kind="ExternalInput")
    ones_d = nc.dram_tensor("ones", [128, DH], f32r, kind="ExternalInput")
    bk_d = nc.dram_tensor("bk", [DC], f32, kind="ExternalInput")
    outT_d = nc.dram_tensor("outT", [C, T], f32, kind="ExternalOutput")

    from contextlib import ExitStack

    with tile.TileContext(nc) as tc, ExitStack() as stack:
        if True:
            persist = stack.enter_context(tc.tile_pool(name="persist", bufs=1))
            psum = stack.enter_context(
                tc.tile_pool(name="psum", bufs=1, space="PSUM")
            )
            # PSUM: tag "sc" 2x[128,1024] (4 banks) + un0..un3 (4 banks) = 8.
            # During attention, the active head owns one PAIR of un tags;
            # deferred (norm part2 / proj) work rotates through the other.
            _unrot = [0]

            def un_tile(name, group=None):
                if group is None:
                    i = _unrot[0] % 4
                    _unrot[0] += 1
                else:
                    base2 = 2 * (group[0] % 2)
                    i = base2 + (group[1][0] % 2)
                    group[1][0] += 1
                return psum.tile([128, CH], f32, name=name, tag=f"un{i}", bufs=1)

            # ---------- loads, first-needed first ----------
            wq_sb, wk_sb, wv_sb, xs = [], [], [], []
            xpool = stack.enter_context(tc.tile_pool(name="xpool", bufs=1))
            for k in range(KT):
                wqk = persist.tile([128, DC], f16, name=f"wq{k}", tag=f"wq{k}")
                nc.sync.dma_start(wqk[:, :], wqT_d.ap()[k * 128:(k + 1) * 128, :])
                wq_sb.append(wqk)
                xk = xpool.tile([128, T], f16, name=f"x{k}", tag=f"x{k}")
                eng = nc.gpsimd if k % 2 else nc.scalar
                eng.dma_start(xk[:, :], xT_d.ap()[k * 128:(k + 1) * 128, :])
                xs.append(xk)
            for k in range(KT):
                wkk = persist.tile([128, DC], f16, name=f"wk{k}", tag=f"wk{k}")
                nc.sync.dma_start(wkk[:, :], wkT_d.ap()[k * 128:(k + 1) * 128, :])
                wk_sb.append(wkk)
            for k in range(KT):
                wvk = persist.tile([128, VE], f16, name=f"wv{k}", tag=f"wv{k}")
                nc.sync.dma_start(wvk[:, :], wvT_d.ap()[k * 128:(k + 1) * 128, :])
                wv_sb.append(wvk)
            bqt, bkt = [], []
            for m in range(DM):
                bqm = persist.tile([128, 1], f32, name=f"bq{m}", tag=f"bq{m}")
                nc.sync.dma_start(
                    bqm[:, :], bq_d.ap()[m * 128:(m + 1) * 128].unsqueeze(1)
                )
                bqt.append(bqm)
                bkm = persist.tile([128, 1], f32, name=f"bk{m}", tag=f"bk{m}")
                nc.sync.dma_start(
                    bkm[:, :], bk_d.ap()[m * 128:(m + 1) * 128].unsqueeze(1)
                )
                bkt.append(bkm)
            wo_sb = []
            for k2 in range(DM):
                wok = persist.tile([128, C], f16, name=f"wo{k2}", tag=f"wo{k2}")
                nc.sync.dma_start(wok[:, :], woT_d.ap()[k2 * 128:(k2 + 1) * 128, :])
                wo_sb.append(wok)

            # key-pad mask bias: -1e30 where x[t,0]==0, plus constant -2
            # shift (exp(s-2) keeps fp16 attn weights < 2k; cancels in norm)
            xc0 = persist.tile([128, NT], f16, name="xc0", tag="xc0")
            nc.sync.dma_start(
                xc0[:, :],
                xT_d.ap()[0:1, :].rearrange("a (t p) -> (a p) t", p=128),
            )
            mb = persist.tile([128, NT], f32, name="mb", tag="mb")
            nc.vector.tensor_scalar(
                out=mb[:, :], in0=xc0[:, :], scalar1=0.0, scalar2=-1e30,
                op0=is_equal, op1=mult,
            )
            nc.vector.tensor_scalar_add(mb[:, :], mb[:, :], -2.0)

            ones64 = persist.tile([1, DH], f32r, name="ones64", tag="ones64")
            nc.sync.dma_start(ones64[:, :], ones_d.ap()[0:1, :])

            # ---------- projections / attention / output ----------
            # Schedule: qk(m0,ch01)+v(0..3) upfront (short PE ramp), then
            # attention rounds with deferred matmul work woven in so the PE
            # never starves (keeps the HAM clock at 2.4 GHz):
            #   h0A: v(4..15)    h1A: qk(m1,ch01)   h2A: qk(m0,ch23)
            #   h3A: qk(m1,ch23) passB: pass-A proj units; proj-B at tail.
            qT = [
                persist.tile([128, T], f16, name=f"qT{m}", tag=f"qT{m}")
                for m in range(DM)
            ]
            kT = [
                persist.tile([128, T], f16, name=f"kT{m}", tag=f"kT{m}")
                for m in range(DM)
            ]

            free_group = [0, [0]]  # [pair index, rotation counter]

            def proj_chain(dst, w_sb, bias, m, ch):
                # one (dst, m, ch) chain: 4 two-matmul sub-units, bias evict
                state = {}
                units = []
                for step in range(4):
                    def u(dst=dst, w_sb=w_sb, bias=bias, ch=ch,
                          state=state, m=m, step=step):
                        if step == 0:
                            state["ps"] = un_tile(
                                f"ps{dst[0].name}{m}{ch}", group=free_group
                            )
                        ps = state["ps"]
                        for k in (2 * step, 2 * step + 1):
                            nc.tensor.matmul(
                                ps[:, :],
                                w_sb[k][:, m * 128:(m + 1) * 128],
                                xs[k][:, ch * CH:(ch + 1) * CH],
                                start=(k == 0),
                                stop=(k == KT - 1),
                            )
                        if step == 3:
                            nc.vector.tensor_scalar_add(
                                dst[m][:, ch * CH:(ch + 1) * CH],
                                ps[:, :],
                                bias[m][:, :],
                            )
                    units.append(u)
                return units

            def q_units(m, chpair):
                out = []
                for ch in (2 * chpair, 2 * chpair + 1):
                    out.extend(proj_chain(qT, wq_sb, bqt, m, ch))
                return out

            def k_units(m):
                out = []
                for ch in range(NCH):
                    out.extend(proj_chain(kT, wk_sb, bkt, m, ch))
                return out

            v_sb = [None] * NT

            def v_subunits(tkt):
                state = {}
                units = []
                for step in range(2):
                    def u(tkt=tkt, state=state, step=step):
                        if step == 0:
                            state["ps"] = un_tile(
                                f"psv{tkt}", group=free_group
                            )
                        psv = state["ps"]
                        for k in range(4 * step, 4 * step + 4):
                            nc.tensor.matmul(
                                psv[:, 0:VE],
                                xs[k][:, tkt * 128:(tkt + 1) * 128],
                                wv_sb[k][:, :],
                                start=(k == 0),
                                stop=(k == KT - 1),
                            )
                        if step == 1:
                            vt = persist.tile(
                                [128, VE], f16, name=f"v{tkt}", tag=f"v{tkt}"
                            )
                            nc.vector.tensor_copy(vt[:, :], psv[:, 0:VE])
                            ones_cols = vt.rearrange(
                                "p (h e) -> p h e", e=DH + 1
                            )[:, :, DH]
                            nc.vector.memset(ones_cols, 1.0)
                            v_sb[tkt] = vt
                    units.append(u)
                return units

            # upfront ramp: qT[0] chunks 0-1 (wq/xs DMAs arrive first),
            # then kT[0] (all chunks), then v(0..3)
            for u in q_units(0, 0):
                u()
            for u in k_units(0):
                u()
            for tkt in range(4):
                for u in v_subunits(tkt):
                    u()

            headsT = [
                persist.tile([128, T], f16, name=f"headsT{m}", tag=f"hT{m}")
                for m in range(DM)
            ]
            atpool = stack.enter_context(tc.tile_pool(name="atpool", bufs=1))
            work = stack.enter_context(tc.tile_pool(name="work", bufs=1))

            pending_norm = []

            def make_norm_part2(h, pas, sub, unev, rr):
                m, base = h // 2, (h % 2) * 64
                ch = pas * 2 + sub

                def emit():
                    rb = un_tile(f"rb{h}p{pas}s{sub}", group=free_group)
                    nc.tensor.matmul(
                        rb[0:DH, :], ones64[:, :], rr[:, :], start=True,
                        stop=True,
                    )
                    if base == 0:
                        nc.vector.tensor_mul(
                            headsT[m][0:DH, ch * CH:(ch + 1) * CH],
                            unev[0:DH, :],
                            rb[0:DH, :],
                        )
                    else:
                        scr = work.tile(
                            [DH, CH], f16, name=f"scr{h}p{pas}{sub}",
                            tag="scr", bufs=4,
                        )
                        nc.vector.tensor_mul(scr[:, :], unev[0:DH, :], rb[0:DH, :])
                        nc.sync.dma_start(
                            headsT[m][base:base + 64, ch * CH:(ch + 1) * CH],
                            scr[:, :],
                        )
                return emit

            def make_proj_unit(mc, ch, parity):
                def emit():
                    pp = un_tile(f"pp{mc}{ch}", group=free_group)
                    for k2 in range(DM):
                        nc.tensor.matmul(
                            pp[:, :],
                            wo_sb[k2][:, mc * 128:(mc + 1) * 128],
                            headsT[k2][:, ch * CH:(ch + 1) * CH],
                            start=(k2 == 0),
                            stop=(k2 == DM - 1),
                        )
                    po = work.tile(
                        [128, CH], f32, name=f"po{mc}{ch}", tag="po", bufs=4
                    )
                    if parity == 0:
                        nc.vector.tensor_copy(po[:, :], pp[:, :])
                    else:
                        nc.scalar.copy(po[:, :], pp[:, :])
                    nc.sync.dma_start(
                        outT_d.ap()[
                            mc * 128:(mc + 1) * 128, ch * CH:(ch + 1) * CH
                        ],
                        po[:, :],
                    )
                return emit

            def proj_chain_units(mc, ch):
                # two sub-units: k2=0 matmul; then k2=1 matmul + DVE evict
                state = {}

                def u0():
                    state["pp"] = un_tile(f"pp{mc}{ch}", group=free_group)
                    nc.tensor.matmul(
                        state["pp"][:, :],
                        wo_sb[0][:, mc * 128:(mc + 1) * 128],
                        headsT[0][:, ch * CH:(ch + 1) * CH],
                        start=True, stop=False,
                    )

                def u1():
                    pp = state["pp"]
                    nc.tensor.matmul(
                        pp[:, :],
                        wo_sb[1][:, mc * 128:(mc + 1) * 128],
                        headsT[1][:, ch * CH:(ch + 1) * CH],
                        start=False, stop=True,
                    )
                    po = work.tile(
                        [128, CH], f32, name=f"po{mc}{ch}", tag="po", bufs=4
                    )
                    nc.vector.tensor_copy(po[:, :], pp[:, :])
                    nc.sync.dma_start(
                        outT_d.ap()[
                            mc * 128:(mc + 1) * 128, ch * CH:(ch + 1) * CH
                        ],
                        po[:, :],
                    )
                return u0, u1

            # round-filler schedule: head_idx (0..7) -> {round t: [closures]}
            filler = {hi: {} for hi in range(2 * HPG)}

            def sched(hi, t, fn):
                filler[hi].setdefault(t, []).append(fn)

            # v(4..15): 24 sub-units into h0A rounds, each v(t) fully
            # emitted by round t-1 (dense early, thins out)
            vunits = []
            for tkt in range(4, NT):
                vunits.extend(v_subunits(tkt))
            for j, u in enumerate(vunits):
                r = min(j * 14 // len(vunits), 13)
                tkt = 4 + j // 2
                r = min(r, tkt - 1)
                sched(0, r, u)
            # h1A: kT[1] (16 units) + qT[1] chunks 0-1 (8 units)
            h1units = k_units(1) + q_units(1, 0)
            for j, u in enumerate(h1units):
                sched(1, j * 16 // len(h1units), u)
            # h2A: qT[0] chunks 2-3 ; h3A: qT[1] chunks 2-3
            for j, u in enumerate(q_units(0, 1)):
                sched(2, j * 2, u)
            for j, u in enumerate(q_units(1, 1)):
                sched(3, j * 2, u)
            # pass-A proj: after all pass-A norms flush (pass-B h0 round
            # 11); single-round units, DVE-only evicts (ACT stays on exp)
            for i in range(16):
                mc, sub = divmod(i, 2)
                if i < 2:
                    sched(4, 12 + 2 * i, make_proj_unit(mc, sub, 0))
                else:
                    j = i - 2
                    sched(5 + j // 5, 2 + 3 * (j % 5),
                          make_proj_unit(mc, sub, 0))

            for pas in range(2):
                for h in range(HPG):
                    hi = pas * HPG + h
                    m, base = h // 2, (h % 2) * 64
                    pair = hi % 2
                    free_group[0] = pair ^ 1
                    un = [
                        un_tile(f"un{h}p{pas}c{s}", group=[pair, [s]])
                        for s in range(2)
                    ]

                    def emit_sc(t, h=h, m=m, base=base, pas=pas):
                        sc = psum.tile(
                            [128, 2 * CH], f32, name=f"sc{h}p{pas}t{t}",
                            tag="sc", bufs=2,
                        )
                        for sub in range(2):
                            ch = pas * 2 + sub
                            nc.tensor.matmul(
                                sc[:, sub * CH:(sub + 1) * CH],
                                kT[m][base:base + 64, t * 128:(t + 1) * 128],
                                qT[m][base:base + 64, ch * CH:(ch + 1) * CH],
                                start=True,
                                stop=True,
                            )
                        return sc

                    def emit_av(ta, ats, h=h):
                        for sub in range(2):
                            nc.tensor.matmul(
                                un[sub][0:DH + 1, :],
                                v_sb[ta][:, h * (DH + 1):(h + 1) * (DH + 1)],
                                ats[:, sub * CH:(sub + 1) * CH],
                                start=(ta == 0),
                                stop=(ta == NT - 1),
                            )

                    av_lag = 0
                    at_hist = {}
                    sc_cur = emit_sc(0)
                    for t in range(NT):
                        at = atpool.tile(
                            [128, 2 * CH], f16, name=f"at{h}p{pas}t{t}",
                            tag="at", bufs=6,
                        )
                        nc.scalar.activation(
                            at[:, :], sc_cur[:, :], Exp, bias=mb[:, t:t + 1]
                        )
                        at_hist[t] = at
                        if t + 1 < NT:
                            sc_cur = emit_sc(t + 1)
                        for fn in filler[hi].get(t, ()):
                            fn()
                        if t - av_lag >= 0:
                            emit_av(t - av_lag, at_hist.pop(t - av_lag))
                        if t in (9, 13) and pending_norm:
                            pending_norm.pop(0)()
                    for ta in sorted(at_hist):
                        emit_av(ta, at_hist.pop(ta))

                    # norm part 1 (PE-free): evict un, extract denom, recip
                    for sub in range(2):
                        unev = work.tile(
                            [DH + 1, CH], f32, name=f"unev{h}p{pas}{sub}",
                            tag="unev", bufs=6,
                        )
                        nc.vector.tensor_copy(unev[:, :], un[sub][0:DH + 1, :])
                        dr = work.tile(
                            [1, CH], f32, name=f"dr{h}p{pas}{sub}", tag="dr",
                            bufs=6,
                        )
                        nc.sync.dma_start(dr[:, :], unev[DH:DH + 1, :])
                        rr = work.tile(
                            [1, CH], f32r, name=f"rr{h}p{pas}{sub}", tag="rr",
                            bufs=6,
                        )
                        with nc.allow_low_precision(reason="fp32r matmul operand"):
                            nc.vector.reciprocal(rr[:, :], dr[:, :])
                        pending_norm.append(
                            make_norm_part2(h, pas, sub, unev, rr)
                        )

            while pending_norm:
                pending_norm.pop(0)()
            # pass-B projection tail: rotate all 4 free PSUM slots
            for i in range(16):
                mc, sub = divmod(i, 2)
                free_group[0] = i % 2
                make_proj_unit(mc, 2 + sub, i % 2)()

    nc.compile()
    return nc

def make_in_maps(x, wq_w, wq_b, wk_w, wk_b, wv_w, wv_b, wo_w, wo_b):
    scale = DH ** -0.5
    in_maps = []
    for c in range(N_CORES):
        b, g = divmod(c, G)
        sl = slice(g * DC, (g + 1) * DC)
        wvT_ext = np.zeros((C, VE), np.float32)
        for hl in range(HPG):
            rows = slice(g * DC + hl * DH, g * DC + (hl + 1) * DH)
            wvT_ext[:, hl * (DH + 1):hl * (DH + 1) + DH] = wv_w[rows, :].T
        in_maps.append({
            "xT": np.ascontiguousarray(x[b].T).astype(np.float16),
            "wqT": np.ascontiguousarray((wq_w[sl] * scale).T).astype(np.float16),
            "wkT": np.ascontiguousarray(wk_w[sl].T).astype(np.float16),
            "wvT": wvT_ext.astype(np.float16),
            "woT": np.ascontiguousarray(wo_w[:, sl].T).astype(np.float16),
            "bq": np.ascontiguousarray(wq_b[sl] * scale).astype(np.float32),
            "ones": np.ones((128, DH), np.float32),
            "bk": np.ascontiguousarray(wk_b[sl]).astype(np.float32),
        })
    return in_maps


def assemble_output(results, wv_b, wo_w, wo_b):
    const_row = wv_b @ wo_w.T + wo_b  # [C]
    out = np.zeros((B, T, C), np.float32)
    for c in range(N_CORES):
        b = c // G
        out[b] += results[c]["outT"].T
    out += const_row[None, None, :]
    return out.astype(np.float32)


_nc_cache = {}


def kernel(**inputs):
    from concourse.bass_utils import run_bass_kernel_spmd

    if "nc" not in _nc_cache:
        _nc_cache["nc"] = build_nc(debug=False)
    nc = _nc_cache["nc"]

    in_maps = make_in_maps(**inputs)
    res = run_bass_kernel_spmd(nc, in_maps, core_ids=list(range(N_CORES)))
    return assemble_output(
        res.results, inputs["wv_b"], inputs["wo_w"], inputs["wo_b"]
    )



# revision 5
# speedup vs baseline: 1.2216x; 1.2216x over previous
"""Trainium2 Bass kernel for 16-head MHA (B=2, T=2048, C=1024).

Sharding: 8 cores = 2 batches x 4 head-groups (4 heads each).
Each core computes, for its batch b and head group g:
  partialT[c, t] = sum_{h in g} wo[:, h].T @ (softmax(qk^T) @ v_h)^T
in fully transposed space (no on-device transposes needed):
  - host passes xT = x[b].T and pre-transposed weight slices
  - qT/kT computed as [d, t]; v as [t, d] (+ ones column per head for the
    softmax denominator); scores computed directly as [tk, tq]
  - exp applied with per-partition (key) mask bias; denominator emerges as
    row 64 of the attn@v_ext matmul output; normalization folded in as a
    K=1 "replicate" matmul + elementwise multiply
  - final projection consumes the [d, t] head outputs as stationary weights
Host adds the 4 partial sums per batch, the wo bias, and the wv_b @ wo.T
constant row (v-bias contribution commutes through softmax normalization).

Schedule: heads are processed in PAIRS (SBUF partition base 0 / 64).  The
two K=64 score matmuls of a pair land on disjoint PE row-groups
(tile_position (0,0) / (64,0)) and execute concurrently, halving score
wall time.  A pass covers one 512-query chunk for both heads of a pair;
the exp runs as one [128, 1024] ACT instruction per key tile, which is
the pacing engine (~1.1us each).  Projection / v / norm work fills the
PE gaps so the HAM clock stays at 2.4 GHz.  Softmax reciprocals are
computed at [128, 4] after a scatter DMA (the [1, 512] layout would use
a single DVE lane at ~8 cycles/element).
"""

import sys

sys.path.insert(0, "/opt/trn_rl_repo")

import numpy as np

# ---- problem constants (hardcoded per harness contract) ----
B = 2
T = 2048
C = 1024
NUM_HEADS = 16
G = 4                 # head groups (tensor-parallel dimension)
HPG = NUM_HEADS // G  # 4 heads per core
DH = C // NUM_HEADS   # 64
DC = HPG * DH         # 256 dims per core
VE = HPG * (DH + 1)   # 260: per head 64 v-dims + 1 ones column
N_CORES = B * G       # 8
PAD_ID = 0.0

CH = 512              # tq chunk (one PSUM bank of fp32)
NCH = T // CH         # 4
NT = T // 128         # 16 token tiles
KT = C // 128         # 8 contraction tiles for projections
DM = DC // 128        # 2 m-tiles for q/k (== head pairs)

# pass order: (pair, chunk).  Chunk-major within a pair-block so that
# kT[1]/qT[1] prep fits in the first two passes and wo-projection for a
# chunk can start two passes after both pairs finish it.
PASS_ORDER = [(0, 0), (0, 1), (1, 0), (1, 1), (0, 2), (0, 3), (1, 2), (1, 3)]


def build_nc(debug=False):
    import concourse.tile as tile
    from concourse import bacc, mybir

    f32 = mybir.dt.float32
    f32r = mybir.dt.float32r
    f16 = mybir.dt.float16
    Exp = mybir.ActivationFunctionType.Exp
    is_equal = mybir.AluOpType.is_equal
    mult = mybir.AluOpType.mult

    nc = bacc.Bacc(
        "TRN2", target_bir_lowering=False, debug=debug, num_devices=N_CORES
    )

    xT_d = nc.dram_tensor("xT", [C, T], f16, kind="ExternalInput")
    wqT_d = nc.dram_tensor("wqT", [C, DC], f16, kind="ExternalInput")
    wkT_d = nc.dram_tensor("wkT", [C, DC], f16, kind="ExternalInput")
    wvT_d = nc.dram_tensor("wvT", [C, VE], f16, kind="ExternalInput")
    woT_d = nc.dram_tensor("woT", [DC, C], f16, kind="ExternalInput")
    bq_d = nc.dram_tensor("bq", [DC], f32, kind="ExternalInput")
    ones_d = nc.dram_tensor("ones", [128, DH], f32r, kind="ExternalInput")
    bk_d = nc.dram_tensor("bk", [DC], f32, kind="ExternalInput")
    outT_d = nc.dram_tensor("outT", [C, T], f32, kind="ExternalOutput")

    from contextlib import ExitStack

    with tile.TileContext(nc) as tc, ExitStack() as stack:
        persist = stack.enter_context(tc.tile_pool(name="persist", bufs=1))
        psum = stack.enter_context(tc.tile_pool(name="psum", bufs=1, space="PSUM"))
        xpool = stack.enter_context(tc.tile_pool(name="xpool", bufs=1))
        atpool = stack.enter_context(tc.tile_pool(name="atpool", bufs=1))
        work = stack.enter_context(tc.tile_pool(name="work", bufs=1))

        # PSUM bank plan (8 banks of [128, 2KB]):
        #   sc  : [128, 1024] f32 x bufs=2  -> 4 banks (score pair tiles)
        #   un0 : [128, 512] f32 x bufs=1   -> 1 bank  (attn@v head 0)
        #   un1 : [128, 512] f32 x bufs=1   -> 1 bank  (attn@v head 1)
        #   pp  : [128, 512] f32 x bufs=2   -> 2 banks (wo proj + norm rb)

        # ---------- loads: k-projection operands first ----------
        xc0 = persist.tile([128, NT], f16, name="xc0", tag="xc0")
        nc.sync.dma_start(
            xc0[:, :],
            xT_d.ap()[0:1, :].rearrange("a (t p) -> (a p) t", p=128),
        )
        wk_sb, wq_sb, wv_sb, xs = [], [], [], []
        for k in range(KT):
            wkk = persist.tile([128, DC], f16, name=f"wk{k}", tag=f"wk{k}")
            nc.sync.dma_start(wkk[:, :], wkT_d.ap()[k * 128:(k + 1) * 128, :])
            wk_sb.append(wkk)
            xk = xpool.tile([128, T], f16, name=f"x{k}", tag=f"x{k}")
            eng = nc.gpsimd if k % 2 else nc.scalar
            eng.dma_start(xk[:, :], xT_d.ap()[k * 128:(k + 1) * 128, :])
            xs.append(xk)
        bqt, bkt = [], []
        for m in range(DM):
            bkm = persist.tile([128, 1], f32, name=f"bk{m}", tag=f"bk{m}")
            nc.sync.dma_start(
                bkm[:, :], bk_d.ap()[m * 128:(m + 1) * 128].unsqueeze(1)
            )
            bkt.append(bkm)
        for k in range(KT):
            wqk = persist.tile([128, DC], f16, name=f"wq{k}", tag=f"wq{k}")
            nc.sync.dma_start(wqk[:, :], wqT_d.ap()[k * 128:(k + 1) * 128, :])
            wq_sb.append(wqk)
        for m in range(DM):
            bqm = persist.tile([128, 1], f32, name=f"bq{m}", tag=f"bq{m}")
            nc.sync.dma_start(
                bqm[:, :], bq_d.ap()[m * 128:(m + 1) * 128].unsqueeze(1)
            )
            bqt.append(bqm)
        for k in range(KT):
            wvk = persist.tile([128, VE], f16, name=f"wv{k}", tag=f"wv{k}")
            nc.sync.dma_start(wvk[:, :], wvT_d.ap()[k * 128:(k + 1) * 128, :])
            wv_sb.append(wvk)
        ones64 = persist.tile([1, DH], f32r, name="ones64", tag="ones64")
        nc.sync.dma_start(ones64[:, :], ones_d.ap()[0:1, :])
        wo_sb = []
        for k2 in range(DM):
            wok = persist.tile([128, C], f16, name=f"wo{k2}", tag=f"wo{k2}")
            nc.sync.dma_start(wok[:, :], woT_d.ap()[k2 * 128:(k2 + 1) * 128, :])
            wo_sb.append(wok)

        # warm the ACT exp table set during the DMA ramp (~2.7us load)
        expwarm = work.tile([1, 1], f32, name="expwarm", tag="expwarm", bufs=1)
        nc.scalar.activation(expwarm[:, :], xc0[0:1, 0:1], Exp)

        # key-pad mask bias: -1e30 where x[t,0]==0, plus constant -2
        # shift (exp(s-2) keeps fp16 attn weights < 2k; cancels in norm)
        mb = persist.tile([128, NT], f32, name="mb", tag="mb")
        nc.vector.tensor_scalar(
            out=mb[:, :], in0=xc0[:, :], scalar1=0.0, scalar2=-1e30,
            op0=is_equal, op1=mult,
        )
        nc.vector.tensor_scalar_add(mb[:, :], mb[:, :], -2.0)

        # ---------- projection / v building blocks ----------
        qT = [
            persist.tile([128, T], f16, name=f"qT{m}", tag=f"qT{m}")
            for m in range(DM)
        ]
        kT = [
            persist.tile([128, T], f16, name=f"kT{m}", tag=f"kT{m}")
            for m in range(DM)
        ]
        headsT = [
            persist.tile([128, T], f16, name=f"headsT{m}", tag=f"hT{m}")
            for m in range(DM)
        ]

        def pp_tile(name):
            return psum.tile([128, CH], f32, name=name, tag="pp", bufs=2)

        def proj_chain(dst, w_sb, bias, m, ch):
            # one (dst, m, ch) chain: 4 two-matmul sub-units, bias evict
            state = {}
            units = []
            for step in range(4):
                def u(dst=dst, w_sb=w_sb, bias=bias, ch=ch,
                      state=state, m=m, step=step):
                    if step == 0:
                        state["ps"] = pp_tile(f"ps{dst[0].name}{m}{ch}")
                    ps = state["ps"]
                    for k in (2 * step, 2 * step + 1):
                        nc.tensor.matmul(
                            ps[:, :],
                            w_sb[k][:, m * 128:(m + 1) * 128],
                            xs[k][:, ch * CH:(ch + 1) * CH],
                            start=(k == 0),
                            stop=(k == KT - 1),
                        )
                    if step == 3:
                        nc.vector.tensor_scalar_add(
                            dst[m][:, ch * CH:(ch + 1) * CH],
                            ps[:, :],
                            bias[m][:, :],
                        )
                units.append(u)
            return units

        def q_units(m, ch):
            return proj_chain(qT, wq_sb, bqt, m, ch)

        def k_units(m):
            out = []
            for ch in range(NCH):
                out.extend(proj_chain(kT, wk_sb, bkt, m, ch))
            return out

        v_sb = [None] * NT

        def v_subunits(tkt):
            state = {}
            units = []
            for step in range(2):
                def u(tkt=tkt, state=state, step=step):
                    if step == 0:
                        state["ps"] = pp_tile(f"psv{tkt}")
                    psv = state["ps"]
                    for k in range(4 * step, 4 * step + 4):
                        nc.tensor.matmul(
                            psv[:, 0:VE],
                            xs[k][:, tkt * 128:(tkt + 1) * 128],
                            wv_sb[k][:, :],
                            start=(k == 0),
                            stop=(k == KT - 1),
                        )
                    if step == 1:
                        vt = persist.tile(
                            [128, VE], f16, name=f"v{tkt}", tag=f"v{tkt}"
                        )
                        nc.vector.tensor_copy(vt[:, :], psv[:, 0:VE])
                        ones_cols = vt.rearrange(
                            "p (h e) -> p h e", e=DH + 1
                        )[:, :, DH]
                        nc.vector.memset(ones_cols, 1.0)
                        v_sb[tkt] = vt
                units.append(u)
            return units

        def make_proj_unit(mc, ch):
            def emit():
                pp = pp_tile(f"pp{mc}{ch}")
                for k2 in range(DM):
                    nc.tensor.matmul(
                        pp[:, :],
                        wo_sb[k2][:, mc * 128:(mc + 1) * 128],
                        headsT[k2][:, ch * CH:(ch + 1) * CH],
                        start=(k2 == 0),
                        stop=(k2 == DM - 1),
                    )
                po = work.tile(
                    [128, CH], f32, name=f"po{mc}{ch}", tag="po", bufs=4
                )
                nc.vector.tensor_copy(po[:, :], pp[:, :])
                nc.sync.dma_start(
                    outT_d.ap()[
                        mc * 128:(mc + 1) * 128, ch * CH:(ch + 1) * CH
                    ],
                    po[:, :],
                )
            return emit

        # ---------- norm (softmax denominator) ----------
        pending_norm = []

        def make_norm_part2(p, ch, hh, unev, rr):
            base = hh * 64

            def emit():
                rb = pp_tile(f"rb{p}{ch}{hh}")
                nc.tensor.matmul(
                    rb[0:DH, :], ones64[:, :], rr[:, :], start=True, stop=True
                )
                if base == 0:
                    nc.vector.tensor_mul(
                        headsT[p][0:DH, ch * CH:(ch + 1) * CH],
                        unev[0:DH, :],
                        rb[0:DH, :],
                    )
                else:
                    scr = work.tile(
                        [DH, CH], f16, name=f"scr{p}{ch}{hh}", tag="scr",
                        bufs=4,
                    )
                    nc.vector.tensor_mul(scr[:, :], unev[0:DH, :], rb[0:DH, :])
                    nc.sync.dma_start(
                        headsT[p][base:base + 64, ch * CH:(ch + 1) * CH],
                        scr[:, :],
                    )
            return emit

        def norm_part1(p, ch, hh, un):
            # evict un, spread denominator row across 128 partitions,
            # reciprocal there, gather back to [1, 512] for the rb matmul
            unev = work.tile(
                [DH + 1, CH], f32, name=f"unev{p}{ch}{hh}", tag="unev", bufs=6
            )
            nc.vector.tensor_copy(unev[:, :], un[0:DH + 1, :])
            drp = work.tile(
                [128, CH // 128], f32, name=f"drp{p}{ch}{hh}", tag="drp",
                bufs=6,
            )
            nc.sync.dma_start(drp[:, :], unev[DH:DH + 1, :])
            rrp = work.tile(
                [128, CH // 128], f32r, name=f"rrp{p}{ch}{hh}", tag="rrp",
                bufs=6,
            )
            with nc.allow_low_precision(reason="fp32r matmul operand"):
                nc.vector.reciprocal(rrp[:, :], drp[:, :])
            rr = work.tile(
                [1, CH], f32r, name=f"rr{p}{ch}{hh}", tag="rr", bufs=6
            )
            nc.sync.dma_start(rr[:, :], rrp[:, :])
            pending_norm.append(make_norm_part2(p, ch, hh, unev, rr))

        # ---------- upfront ramp ----------
        for u in k_units(0):
            u()
        for u in q_units(0, 0):
            u()
        for tkt in range(6):
            for u in v_subunits(tkt):
                u()

        # ---------- filler schedule: pass idx -> {round: [closures]} ----------
        filler = {pi: {} for pi in range(len(PASS_ORDER))}

        def sched(pi, t, fn):
            filler[pi].setdefault(t, []).append(fn)

        # pass 0: v(6..15), each v(t) fully emitted by round t-1
        vunits = []
        for tkt in range(6, NT):
            vunits.extend(v_subunits(tkt))
        for j, u in enumerate(vunits):
            tkt = 6 + j // 2
            sched(0, min(j * 16 // len(vunits), tkt - 1), u)
        # pass 0 also preps qT[0] chunk 1 (needed by pass 1 = (p0, c1))
        for j, u in enumerate(q_units(0, 1)):
            sched(0, 12 + j, u)
        # pass 1: kT[1] (16 units) + qT[1] chunk 0 (4 units)
        p1u = k_units(1) + q_units(1, 0)
        for j, u in enumerate(p1u):
            sched(1, j * 16 // len(p1u), u)
        # pass 2: qT[1] chunk 1, qT[0] chunk 2
        for j, u in enumerate(q_units(1, 1)):
            sched(2, 8 + j, u)
        for j, u in enumerate(q_units(0, 2)):
            sched(2, 12 + j, u)
        # pass 3: wo-proj chunk 0 (after p1c0 norm part2 at rounds 0/2) + q(1,2)
        for i in range(8):
            sched(3, 3 + i, make_proj_unit(i, 0))
        for j, u in enumerate(q_units(1, 2)):
            sched(3, 12 + j, u)
        # pass 4: wo-proj chunk 1 + q(0,3)
        for i in range(8):
            sched(4, 3 + i, make_proj_unit(i, 1))
        for j, u in enumerate(q_units(0, 3)):
            sched(4, 12 + j, u)
        # pass 5: q(1,3)
        for j, u in enumerate(q_units(1, 3)):
            sched(5, 4 + 2 * j, u)
        # pass 7: wo-proj chunk 2
        for i in range(8):
            sched(7, 3 + i, make_proj_unit(i, 2))

        # norm part2 pops: two per pass starting at pass 1 (pass i's norms
        # are popped early in pass i+1; pass 0's in pass 2 to keep pass 1
        # free for kT[1]/qT[1] prep)
        pops = {2: (1, 3, 5, 7), 3: (0, 2), 4: (0, 2), 5: (1, 3),
                6: (1, 3), 7: (0, 2)}

        # ---------- attention passes ----------
        for pi, (p, ch) in enumerate(PASS_ORDER):
            un = [
                psum.tile([128, CH], f32, name=f"un{p}{ch}{hh}",
                          tag=f"un{hh}", bufs=1)
                for hh in range(2)
            ]

            def emit_sc(t, p=p, ch=ch):
                sc = psum.tile(
                    [128, 2 * CH], f32, name=f"sc{p}{ch}t{t}", tag="sc",
                    bufs=2,
                )
                for hh in range(2):
                    base = hh * 64
                    nc.tensor.matmul(
                        sc[:, hh * CH:(hh + 1) * CH],
                        kT[p][base:base + 64, t * 128:(t + 1) * 128],
                        qT[p][base:base + 64, ch * CH:(ch + 1) * CH],
                        start=True,
                        stop=True,
                    )
                return sc

            def emit_av(ta, ats, p=p, un=un):
                for hh in range(2):
                    h = 2 * p + hh
                    nc.tensor.matmul(
                        un[hh][0:DH + 1, :],
                        v_sb[ta][:, h * (DH + 1):(h + 1) * (DH + 1)],
                        ats[:, hh * CH:(hh + 1) * CH],
                        start=(ta == 0),
                        stop=(ta == NT - 1),
                    )

            sc_cur = emit_sc(0)
            for t in range(NT):
                at = atpool.tile(
                    [128, 2 * CH], f16, name=f"at{p}{ch}t{t}", tag="at",
                    bufs=6,
                )
                nc.scalar.activation(
                    at[:, :], sc_cur[:, :], Exp, bias=mb[:, t:t + 1]
                )
                if t + 1 < NT:
                    sc_cur = emit_sc(t + 1)
                for fn in filler[pi].get(t, ()):
                    fn()
                if t in pops.get(pi, ()) and pending_norm:
                    pending_norm.pop(0)()
                emit_av(t, at)

            for hh in range(2):
                norm_part1(p, ch, hh, un[hh])

        # ---------- tail: last norms + wo-proj chunk 3 ----------
        while pending_norm:
            pending_norm.pop(0)()
        for i in range(8):
            make_proj_unit(i, 3)()

    nc.compile()
    return nc


def make_in_maps(x, wq_w, wq_b, wk_w, wk_b, wv_w, wv_b, wo_w, wo_b):
    scale = DH ** -0.5
    in_maps = []
    for c in range(N_CORES):
        b, g = divmod(c, G)
        sl = slice(g * DC, (g + 1) * DC)
        wvT_ext = np.zeros((C, VE), np.float32)
        for hl in range(HPG):
            rows = slice(g * DC + hl * DH, g * DC + (hl + 1) * DH)
            wvT_ext[:, hl * (DH + 1):hl * (DH + 1) + DH] = wv_w[rows, :].T
        in_maps.append({
            "xT": np.ascontiguousarray(x[b].T).astype(np.float16),
            "wqT": np.ascontiguousarray((wq_w[sl] * scale).T).astype(np.float16),
            "wkT": np.ascontiguousarray(wk_w[sl].T).astype(np.float16),
            "wvT": wvT_ext.astype(np.float16),
            "woT": np.ascontiguousarray(wo_w[:, sl].T).astype(np.float16),
            "bq": np.ascontiguousarray(wq_b[sl] * scale).astype(np.float32),
            "ones": np.ones((128, DH), np.float32),
            "bk": np.ascontiguousarray(wk_b[sl]).astype(np.float32),
        })
    return in_maps


def assemble_output(results, wv_b, wo_w, wo_b):
    const_row = wv_b @ wo_w.T + wo_b  # [C]
    out = np.zeros((B, T, C), np.float32)
    for c in range(N_CORES):
        b = c // G
        out[b] += results[c]["outT"].T
    out += const_row[None, None, :]
    return out.astype(np.float32)


_nc_cache = {}


def kernel(**inputs):
    from concourse.bass_utils import run_bass_kernel_spmd

    if "nc" not in _nc_cache:
        _nc_cache["nc"] = build_nc(debug=False)
    nc = _nc_cache["nc"]

    in_maps = make_in_maps(**inputs)
    res = run_bass_kernel_spmd(nc, in_maps, core_ids=list(range(N_CORES)))
    return assemble_output(
        res.results, inputs["wv_b"], inputs["wo_w"], inputs["wo_b"]
    )


# revision 9
# speedup vs baseline: 1.2235x; 1.0016x over previous
"""Trainium2 Bass kernel for 16-head MHA (B=2, T=2048, C=1024).

Sharding: 8 cores = 2 batches x 4 head-groups (4 heads each).
Each core computes, for its batch b and head group g:
  partialT[c, t] = sum_{h in g} wo[:, h].T @ (softmax(qk^T) @ v_h)^T
in fully transposed space (no on-device transposes needed):
  - host passes xT = x[b].T, pre-transposed weight slices, and the
    key-pad mask bias (-1e30 on pad keys, -2 shift) as [128, 16]
  - qT/kT computed as [d, t]; v as [t, d] (+ ones column per head for the
    softmax denominator); scores computed directly as [tk, tq]
  - denominator emerges as row 64 of the attn@v_ext matmul output;
    normalization folded in as a K=1 "replicate" matmul + multiply
  - final projection consumes the [d, t] head outputs as stationary weights
Host adds the 4 partial sums per batch, the wo bias, and the wv_b @ wo.T
constant row (v-bias contribution commutes through softmax normalization).

Schedule: heads are processed in PAIRS (SBUF partition base 0 / 64).  The
two K=64 score matmuls of a pair land on disjoint PE row-groups
(tile_position (0,0) / (64,0)) and execute concurrently, halving score
wall time.  A pass covers one 512-query chunk for both heads of a pair;
the exp runs as one [128, 1024] ACT instruction per key tile, which is
the pacing engine (~1.1us each).  Projection / v / norm work fills the
PE slack per round so the HAM clock stays at 2.4 GHz.  Softmax
reciprocals run at [128, 4] after a scatter DMA (a [1, 512] layout would
use one DVE lane at ~8 cycles/element).  x loads spread over three DMA
queues; only k(0,c0)+q(0,0) precede the first score tile.
"""

import sys

sys.path.insert(0, "/opt/trn_rl_repo")

import numpy as np

# ---- problem constants (hardcoded per harness contract) ----
B = 2
T = 2048
C = 1024
NUM_HEADS = 16
G = 4                 # head groups (tensor-parallel dimension)
HPG = NUM_HEADS // G  # 4 heads per core
DH = C // NUM_HEADS   # 64
DC = HPG * DH         # 256 dims per core
VE = HPG * (DH + 1)   # 260: per head 64 v-dims + 1 ones column
N_CORES = B * G       # 8
PAD_ID = 0.0

CH = 512              # tq chunk (one PSUM bank of fp32)
NCH = T // CH         # 4
NT = T // 128         # 16 token tiles
KT = C // 128         # 8 contraction tiles for projections
DM = DC // 128        # 2 m-tiles for q/k (== head pairs)

PASS_ORDER = [(0, 0), (0, 1), (1, 0), (1, 1), (0, 2), (0, 3), (1, 2), (1, 3)]


def build_nc(debug=False):
    import concourse.tile as tile
    from concourse import bacc, mybir

    f32 = mybir.dt.float32
    f32r = mybir.dt.float32r
    f16 = mybir.dt.float16
    Exp = mybir.ActivationFunctionType.Exp

    nc = bacc.Bacc(
        "TRN2", target_bir_lowering=False, debug=debug, num_devices=N_CORES
    )

    xT_d = nc.dram_tensor("xT", [C, T], f16, kind="ExternalInput")
    wqT_d = nc.dram_tensor("wqT", [C, DC], f16, kind="ExternalInput")
    wkT_d = nc.dram_tensor("wkT", [C, DC], f16, kind="ExternalInput")
    wvT_d = nc.dram_tensor("wvT", [C, VE], f16, kind="ExternalInput")
    woT_d = nc.dram_tensor("woT", [DC, C], f16, kind="ExternalInput")
    bq_d = nc.dram_tensor("bq", [DC], f32, kind="ExternalInput")
    ones_d = nc.dram_tensor("ones", [128, DH], f32r, kind="ExternalInput")
    bk_d = nc.dram_tensor("bk", [DC], f32, kind="ExternalInput")
    mbias_d = nc.dram_tensor("mbias", [128, NT], f32, kind="ExternalInput")
    outT_d = nc.dram_tensor("outT", [C, T], f16, kind="ExternalOutput")

    from contextlib import ExitStack

    with tile.TileContext(nc) as tc, ExitStack() as stack:
        persist = stack.enter_context(tc.tile_pool(name="persist", bufs=1))
        psum = stack.enter_context(tc.tile_pool(name="psum", bufs=1, space="PSUM"))
        xpool = stack.enter_context(tc.tile_pool(name="xpool", bufs=1))
        atpool = stack.enter_context(tc.tile_pool(name="atpool", bufs=1))
        work = stack.enter_context(tc.tile_pool(name="work", bufs=1))

        # PSUM bank plan (8 banks of [128, 2KB]):
        #   sc  : [128, 1024] f32 x bufs=2  -> 4 banks (score pair tiles)
        #   un0 : [128, 512] f32 x bufs=1   -> 1 bank  (attn@v head 0)
        #   un1 : [128, 512] f32 x bufs=1   -> 1 bank  (attn@v head 1)
        #   pp  : [128, 512] f32 x bufs=2   -> 2 banks (q/k/v/wo proj + rb)

        # ---------- loads ----------
        # sync queue: mask bias, then weights in first-use order
        mb = persist.tile([128, NT], f32, name="mb", tag="mb")
        nc.sync.dma_start(mb[:, :], mbias_d.ap()[:, :])
        wk_sb, wq_sb, wv_sb = [], [], []
        for k in range(KT):
            wkk = persist.tile([128, DC], f16, name=f"wk{k}", tag=f"wk{k}")
            nc.sync.dma_start(wkk[:, :], wkT_d.ap()[k * 128:(k + 1) * 128, :])
            wk_sb.append(wkk)
        bkt, bqt = [], []
        for m in range(DM):
            bkm = persist.tile([128, 1], f32, name=f"bk{m}", tag=f"bk{m}")
            nc.sync.dma_start(
                bkm[:, :], bk_d.ap()[m * 128:(m + 1) * 128].unsqueeze(1)
            )
            bkt.append(bkm)
        for k in range(KT):
            wqk = persist.tile([128, DC], f16, name=f"wq{k}", tag=f"wq{k}")
            nc.sync.dma_start(wqk[:, :], wqT_d.ap()[k * 128:(k + 1) * 128, :])
            wq_sb.append(wqk)
        for m in range(DM):
            bqm = persist.tile([128, 1], f32, name=f"bq{m}", tag=f"bq{m}")
            nc.sync.dma_start(
                bqm[:, :], bq_d.ap()[m * 128:(m + 1) * 128].unsqueeze(1)
            )
            bqt.append(bqm)
        for k in range(KT):
            wvk = persist.tile([128, VE], f16, name=f"wv{k}", tag=f"wv{k}")
            nc.sync.dma_start(wvk[:, :], wvT_d.ap()[k * 128:(k + 1) * 128, :])
            wv_sb.append(wvk)
        ones64 = persist.tile([1, DH], f32r, name="ones64", tag="ones64")
        nc.sync.dma_start(ones64[:, :], ones_d.ap()[0:1, :])
        wo_sb = []
        for k2 in range(DM):
            wok = persist.tile([128, C], f16, name=f"wo{k2}", tag=f"wo{k2}")
            nc.sync.dma_start(wok[:, :], woT_d.ap()[k2 * 128:(k2 + 1) * 128, :])
            wo_sb.append(wok)

        # x tiles: two queues, column-halves so chunk-0/1 consumers (first
        # pass, first q/k chunks, v tiles 0-7) unblock before the full load
        xs = [None] * KT
        for k in range(KT):
            xs[k] = xpool.tile([128, T], f16, name=f"x{k}", tag=f"x{k}")
        HT = T // 2
        for half in range(2):
            for k in range(KT):
                eng = nc.gpsimd if k % 2 else nc.scalar
                eng.dma_start(
                    xs[k][:, half * HT:(half + 1) * HT],
                    xT_d.ap()[k * 128:(k + 1) * 128, half * HT:(half + 1) * HT],
                )

        # warm the ACT exp table set (~2.7us) behind the x descriptors
        expwarm = work.tile([1, 1], f32, name="expwarm", tag="expwarm", bufs=1)
        nc.vector.memset(expwarm[:, :], 0.0)
        nc.scalar.activation(expwarm[:, :], expwarm[:, :], Exp)

        # ---------- projection / v building blocks ----------
        qT = [
            persist.tile([128, T], f16, name=f"qT{m}", tag=f"qT{m}")
            for m in range(DM)
        ]
        kT = [
            persist.tile([128, T], f16, name=f"kT{m}", tag=f"kT{m}")
            for m in range(DM)
        ]
        headsT = [
            persist.tile([128, T], f16, name=f"headsT{m}", tag=f"hT{m}")
            for m in range(DM)
        ]

        def pp_tile(name):
            return psum.tile([128, CH], f32, name=name, tag="pp", bufs=2)

        def proj_chain(dst, w_sb, bias, m, ch):
            # one (dst, m, ch) chain: 4 two-matmul sub-units, bias evict
            state = {}
            units = []
            for step in range(4):
                def u(dst=dst, w_sb=w_sb, bias=bias, ch=ch,
                      state=state, m=m, step=step):
                    if step == 0:
                        state["ps"] = pp_tile(f"ps{dst[0].name}{m}{ch}")
                    ps = state["ps"]
                    for k in (2 * step, 2 * step + 1):
                        nc.tensor.matmul(
                            ps[:, :],
                            w_sb[k][:, m * 128:(m + 1) * 128],
                            xs[k][:, ch * CH:(ch + 1) * CH],
                            start=(k == 0),
                            stop=(k == KT - 1),
                        )
                    if step == 3:
                        nc.vector.tensor_scalar_add(
                            dst[m][:, ch * CH:(ch + 1) * CH],
                            ps[:, :],
                            bias[m][:, :],
                        )
                units.append(u)
            return units

        def q_units(m, ch):
            return proj_chain(qT, wq_sb, bqt, m, ch)

        def k_units(m, ch):
            return proj_chain(kT, wk_sb, bkt, m, ch)

        v_sb = [None] * NT

        def v_subunits(tkt):
            state = {}
            units = []
            for step in range(2):
                def u(tkt=tkt, state=state, step=step):
                    if step == 0:
                        state["ps"] = pp_tile(f"psv{tkt}")
                    psv = state["ps"]
                    for k in range(4 * step, 4 * step + 4):
                        nc.tensor.matmul(
                            psv[:, 0:VE],
                            xs[k][:, tkt * 128:(tkt + 1) * 128],
                            wv_sb[k][:, :],
                            start=(k == 0),
                            stop=(k == KT - 1),
                        )
                    if step == 1:
                        vt = persist.tile(
                            [128, VE], f16, name=f"v{tkt}", tag=f"v{tkt}"
                        )
                        nc.vector.tensor_copy(vt[:, :], psv[:, 0:VE])
                        ones_cols = vt.rearrange(
                            "p (h e) -> p h e", e=DH + 1
                        )[:, :, DH]
                        nc.vector.memset(ones_cols, 1.0)
                        v_sb[tkt] = vt
                units.append(u)
            return units

        def proj_units(mc, ch):
            # wo-projection for output rows [mc*128, ...) of chunk ch,
            # split into two ~512-cycle halves
            state = {}

            def u0():
                state["pp"] = pp_tile(f"pp{mc}{ch}")
                nc.tensor.matmul(
                    state["pp"][:, :],
                    wo_sb[0][:, mc * 128:(mc + 1) * 128],
                    headsT[0][:, ch * CH:(ch + 1) * CH],
                    start=True, stop=False,
                )

            def u1():
                pp = state["pp"]
                nc.tensor.matmul(
                    pp[:, :],
                    wo_sb[1][:, mc * 128:(mc + 1) * 128],
                    headsT[1][:, ch * CH:(ch + 1) * CH],
                    start=False, stop=True,
                )
                po = work.tile(
                    [128, CH], f16, name=f"po{mc}{ch}", tag="po", bufs=4
                )
                nc.vector.tensor_copy(po[:, :], pp[:, :])
                nc.sync.dma_start(
                    outT_d.ap()[
                        mc * 128:(mc + 1) * 128, ch * CH:(ch + 1) * CH
                    ],
                    po[:, :],
                )
            return u0, u1

        # ---------- norm (softmax denominator) ----------
        pending_norm = []

        def make_norm_part2(p, ch, hh, unev, rr):
            base = hh * 64

            def emit():
                rb = pp_tile(f"rb{p}{ch}{hh}")
                nc.tensor.matmul(
                    rb[0:DH, :], ones64[:, :], rr[:, :], start=True, stop=True
                )
                if base == 0:
                    nc.vector.tensor_mul(
                        headsT[p][0:DH, ch * CH:(ch + 1) * CH],
                        unev[0:DH, :],
                        rb[0:DH, :],
                    )
                else:
                    scr = work.tile(
                        [DH, CH], f16, name=f"scr{p}{ch}{hh}", tag="scr",
                        bufs=4,
                    )
                    nc.vector.tensor_mul(scr[:, :], unev[0:DH, :], rb[0:DH, :])
                    nc.sync.dma_start(
                        headsT[p][base:base + 64, ch * CH:(ch + 1) * CH],
                        scr[:, :],
                    )
            return emit

        def norm_part1(p, ch, hh, un):
            # evict un, spread denominator row across 128 partitions,
            # reciprocal there, gather back to [1, 512] for the rb matmul
            unev = work.tile(
                [DH + 1, CH], f32, name=f"unev{p}{ch}{hh}", tag="unev", bufs=6
            )
            nc.vector.tensor_copy(unev[:, :], un[0:DH + 1, :])
            drp = work.tile(
                [128, CH // 128], f32, name=f"drp{p}{ch}{hh}", tag="drp",
                bufs=6,
            )
            nc.sync.dma_start(drp[:, :], unev[DH:DH + 1, :])
            rrp = work.tile(
                [128, CH // 128], f32r, name=f"rrp{p}{ch}{hh}", tag="rrp",
                bufs=6,
            )
            with nc.allow_low_precision(reason="fp32r matmul operand"):
                nc.vector.reciprocal(rrp[:, :], drp[:, :])
            rr = work.tile(
                [1, CH], f32r, name=f"rr{p}{ch}{hh}", tag="rr", bufs=6
            )
            nc.sync.dma_start(rr[:, :], rrp[:, :])
            pending_norm.append(make_norm_part2(p, ch, hh, unev, rr))

        # ---------- minimal ramp: just enough for sc(0) of pass 0 ----------
        for u in k_units(0, 0):
            u()
        for u in q_units(0, 0):
            u()

        # ---------- filler schedule: pass idx -> {round: [closures]} ----------
        filler = {pi: {} for pi in range(len(PASS_ORDER))}

        def sched(pi, t, *fns):
            filler[pi].setdefault(t, []).extend(fns)

        vu = {t: v_subunits(t) for t in range(NT)}
        # pass 0: v(0..15) [deadline: round t], kT[0] c1/c2/c3
        # [deadline: round 4c-2], qT[0] c1 [deadline: end of pass]
        k01, k02, k03 = k_units(0, 1), k_units(0, 2), k_units(0, 3)
        sched(0, 0, *vu[0], *vu[1])
        sched(0, 1, *vu[2], *k01[0:2])
        sched(0, 2, *vu[3], *k01[2:4])
        sched(0, 3, *vu[4])
        sched(0, 4, *vu[5], *k02[0:2])
        sched(0, 5, *vu[6], *k02[2:4])
        sched(0, 6, *vu[7])
        sched(0, 7, *vu[8])
        sched(0, 8, *vu[9], *k03[0:2])
        sched(0, 9, *vu[10], *k03[2:4])
        sched(0, 10, *vu[11])
        sched(0, 11, *vu[12])
        q01 = q_units(0, 1)
        sched(0, 12, *vu[13], q01[0])
        sched(0, 13, *vu[14], q01[1])
        sched(0, 14, *vu[15], q01[2])
        sched(0, 15, q01[3])
        # hold the k chain units of pass 0 out of the round-1/2 lists when
        # the chunk chain would overlap an open v chain: layout above keeps
        # at most two pp chains open at any point.

        # pass 1: kT[1] c0/c1 + qT[1] c0; kT[1] c2/c3 slide into pass 2
        # (pass 2 reads chunk c at sc(4c), emitted round 4c-1)
        for c in range(2):
            ku = k_units(1, c)
            sched(1, 2 * c, *ku[0:2])
            sched(1, 2 * c + 1, *ku[2:4])
        q10 = q_units(1, 0)
        for j in range(4):
            sched(1, 4 + 3 * j, q10[j])
        k12, k13 = k_units(1, 2), k_units(1, 3)
        sched(2, 0, *k12[0:2])
        sched(2, 1, *k12[2:4])
        sched(2, 4, *k13[0:2])
        sched(2, 5, *k13[2:4])
        # pass 2: qT[1] c1, qT[0] c2
        for j, u in enumerate(q_units(1, 1)):
            sched(2, 8 + j, u)
        for j, u in enumerate(q_units(0, 2)):
            sched(2, 12 + j, u)
        # passes 3/4/7: wo-projection for chunks 0/1/2 (halves pipelined)
        for pi, ch in ((3, 0), (4, 1), (7, 2)):
            prev = None
            for mc in range(8):
                u0, u1 = proj_units(mc, ch)
                if prev is None:
                    sched(pi, 3, u0)
                else:
                    sched(pi, 3 + mc, prev, u0)
                prev = u1
            sched(pi, 11, prev)
        for j, u in enumerate(q_units(1, 2)):
            sched(3, 12 + j, u)
        for j, u in enumerate(q_units(0, 3)):
            sched(4, 12 + j, u)
        for j, u in enumerate(q_units(1, 3)):
            sched(5, 8 + 2 * j, u)

        # norm part2 pops: pass i's norms pop early in pass i+1 (pass 0's
        # and 1's in pass 2, which is otherwise light)
        pops = {2: (1, 3, 5, 7), 3: (0, 2), 4: (0, 2), 5: (1, 3),
                6: (1, 3), 7: (0, 2)}

        # ---------- attention passes ----------
        for pi, (p, ch) in enumerate(PASS_ORDER):
            un = [
                psum.tile([128, CH], f32, name=f"un{p}{ch}{hh}",
                          tag=f"un{hh}", bufs=1)
                for hh in range(2)
            ]

            def emit_sc(t, p=p, ch=ch):
                sc = psum.tile(
                    [128, 2 * CH], f32, name=f"sc{p}{ch}t{t}", tag="sc",
                    bufs=2,
                )
                for hh in range(2):
                    base = hh * 64
                    nc.tensor.matmul(
                        sc[:, hh * CH:(hh + 1) * CH],
                        kT[p][base:base + 64, t * 128:(t + 1) * 128],
                        qT[p][base:base + 64, ch * CH:(ch + 1) * CH],
                        start=True,
                        stop=True,
                    )
                return sc

            def emit_av(ta, ats, hh, p=p, un=un):
                h = 2 * p + hh
                nc.tensor.matmul(
                    un[hh][0:DH + 1, :],
                    v_sb[ta][:, h * (DH + 1):(h + 1) * (DH + 1)],
                    ats[:, hh * CH:(hh + 1) * CH],
                    start=(ta == 0),
                    stop=(ta == NT - 1),
                )

            sc_cur = emit_sc(0)
            for t in range(NT):
                at = atpool.tile(
                    [128, 2 * CH], f16, name=f"at{p}{ch}t{t}", tag="at",
                    bufs=6,
                )
                nc.scalar.activation(
                    at[:, :], sc_cur[:, :], Exp, bias=mb[:, t:t + 1]
                )
                if t + 1 < NT:
                    sc_cur = emit_sc(t + 1)
                for fn in filler[pi].get(t, ()):
                    fn()
                if t in pops.get(pi, ()) and pending_norm:
                    pending_norm.pop(0)()
                emit_av(t, at, 0)
                emit_av(t, at, 1)

            for hh in range(2):
                norm_part1(p, ch, hh, un[hh])

        # ---------- tail: last norms + wo-proj chunk 3 ----------
        while pending_norm:
            pending_norm.pop(0)()
        for mc in range(8):
            u0, u1 = proj_units(mc, 3)
            u0()
            u1()

    nc.compile()
    return nc


def make_in_maps(x, wq_w, wq_b, wk_w, wk_b, wv_w, wv_b, wo_w, wo_b):
    scale = DH ** -0.5
    in_maps = []
    for c in range(N_CORES):
        b, g = divmod(c, G)
        sl = slice(g * DC, (g + 1) * DC)
        wvT_ext = np.zeros((C, VE), np.float32)
        for hl in range(HPG):
            rows = slice(g * DC + hl * DH, g * DC + (hl + 1) * DH)
            wvT_ext[:, hl * (DH + 1):hl * (DH + 1) + DH] = wv_w[rows, :].T
        # key-pad mask bias [128, NT]: -1e30 on pad keys, -2 everywhere
        col0 = np.asarray(x[b][:, 0], np.float32).reshape(NT, 128).T
        mbias = np.where(col0 == PAD_ID, -1e30, 0.0).astype(np.float32) - 2.0
        in_maps.append({
            "xT": np.ascontiguousarray(x[b].T).astype(np.float16),
            "wqT": np.ascontiguousarray((wq_w[sl] * scale).T).astype(np.float16),
            "wkT": np.ascontiguousarray(wk_w[sl].T).astype(np.float16),
            "wvT": wvT_ext.astype(np.float16),
            "woT": np.ascontiguousarray(wo_w[:, sl].T).astype(np.float16),
            "bq": np.ascontiguousarray(wq_b[sl] * scale).astype(np.float32),
            "ones": np.ones((128, DH), np.float32),
            "bk": np.ascontiguousarray(wk_b[sl]).astype(np.float32),
            "mbias": np.ascontiguousarray(mbias),
        })
    return in_maps


def assemble_output(results, wv_b, wo_w, wo_b):
    const_row = wv_b @ wo_w.T + wo_b  # [C]
    out = np.zeros((B, T, C), np.float32)
    for c in range(N_CORES):
        b = c // G
        out[b] += results[c]["outT"].astype(np.float32).T
    out += const_row[None, None, :]
    return out.astype(np.float32)


_nc_cache = {}


def kernel(**inputs):
    from concourse.bass_utils import run_bass_kernel_spmd

    if "nc" not in _nc_cache:
        _nc_cache["nc"] = build_nc(debug=False)
    nc = _nc_cache["nc"]

    in_maps = make_in_maps(**inputs)
    res = run_bass_kernel_spmd(nc, in_maps, core_ids=list(range(N_CORES)))
    return assemble_output(
        res.results, inputs["wv_b"], inputs["wo_w"], inputs["wo_b"]
    )
